# revision 11
# baseline (speedup 1.0000x reference)
"""ConnectionProductBlock on 8 TRN2 NeuronCores.

out[b, c*K + k, h, w] = am_out[b, c, h, w] * first_out[b, k, h, w]
  with B=16, C=8, K=64, H=W=56.

Data parallel over batch, 2 batches per core, no communication.

v1 strategy (vs v0's fp32 pipeline): the correctness budget (rel err
2e-2 L2) is far looser than fp32, so the whole hot path runs in bf16:
  - Inputs host-cast to bf16; output written as bf16 and host-upcast to
    fp32. Halves HBM/SBUF-port traffic (14.6MB -> 7.4MB per core).
  - am is broadcast across each batch's 64 k-partitions by a selector
    matmul on the TensorEngine into PSUM (fp32), 392-col chunks at
    512-aligned offsets so 4 chunks = 4 PSUM banks = one [128,1568]
    half-unit; 2 PSUM bufs double-buffer PE against consumers.
  - Per half-unit, one of two paths turns PSUM rep + bf16 first into
    bf16 out:
      'B': ScalarE copies rep PSUM->SBUF bf16 (~(172+1568)/1.2 ns),
           then VectorE multiplies all-bf16-SBUF at 2x (~(58+784)/.96).
      'A': VectorE multiplies straight from PSUM at 1x (~(120+1568)/.96).
    Mixing B and A balances ACT and DVE busy time (~17-18us each, vs
    33us DVE-bound in v0).
  - A tiny ScalarE warm-up copy is issued before everything so the
    one-time activation-table load overlaps the input DMAs.
  - All output DMAs ride the SP HWDGE ring as [128,1568] transfers
    (full 16-port engagement per DMA).
"""

import numpy as np

B, C, K, H, W = 16, 8, 64, 56, 56
HW = H * W  # 3136
NCORES = 8
BPC = B // NCORES  # 2
NQ = 4  # quarter-units per c
FDQ = HW // NQ  # 784
NCH = 2  # PSUM chunks (banks) per quarter-unit
CH = FDQ // NCH  # 392
BANK = 512  # fp32 slots per PSUM bank

# One entry per (c, quarter) unit: 'B' = ACT copy + DVE 2x; 'A' = DVE 1x.
# 2-bank PSUM tiles / 4 bufs keep the PE 2-3 units ahead so consumers
# run back-to-back; every 4th unit on path A balances ACT vs DVE.
DEFAULT_PATHS = tuple("A" if u % 4 == 3 else "B" for u in range(C * NQ))

_PROGRAMS = {}


def _build_program(repeat=1, paths=DEFAULT_PATHS):
    import contextlib

    import concourse.bacc as bacc
    import concourse.mybir as mybir
    import concourse.tile as tile

    nc = bacc.Bacc("TRN2", debug=False)
    # am bf16 plane with per-c selector blocks appended on the free dim.
    # Partition = b*8 + c. sel[b*8+c, c*128 + b*64 + k] = 1, so
    # sel_c.T @ am3 writes rep[b*64+k, f] = am[b, c, f].
    amsel = nc.dram_tensor(
        "amsel", [BPC * C, HW + C * 128], mybir.dt.bfloat16, kind="ExternalInput"
    )
    first = nc.dram_tensor(
        "first", [BPC * K, HW], mybir.dt.bfloat16, kind="ExternalInput"
    )
    # c-major so each (c, half) DMA is one [128, FDH] slice with adjacent
    # (b, k) partition strides; host transposes back to [BPC, C*K, HW].
    out = nc.dram_tensor(
        "out", [C, BPC * K, HW], mybir.dt.bfloat16, kind="ExternalOutput"
    )

    with tile.TileContext(nc) as tc:
        with (
            tc.tile_pool(name="ins", bufs=1) as ins_pool,
            tc.tile_pool(name="rep", bufs=4, space="PSUM") as psum_pool,
            tc.tile_pool(name="rsb", bufs=4) as rsb_pool,
            tc.tile_pool(name="outs", bufs=3) as out_pool,
            tc.For_i(0, repeat, 1) if repeat > 1 else contextlib.nullcontext(),
        ):
            # ACT warm-up: the first ACTIVATE triggers the ~2.7us
            # activation-table load; spend it on 2 dummy elements while
            # the input DMAs are still in flight.
            warm_a = ins_pool.tile([1, 2], mybir.dt.float32, tag="warm_a")
            warm_b = ins_pool.tile([1, 2], mybir.dt.float32, tag="warm_b")
            nc.vector.memset(warm_a[:], 0.0)
            nc.scalar.copy(warm_b[:], warm_a[:])

            am3 = ins_pool.tile([BPC * C, HW + C * 128], mybir.dt.bfloat16, tag="am3")
            first2 = ins_pool.tile([BPC * K, HW], mybir.dt.bfloat16, tag="first2")
            nc.sync.dma_start(out=am3[:], in_=amsel.ap())
            nc.sync.dma_start(out=first2[:], in_=first.ap())

            out_ap = out.ap()
            for c in range(C):
                # One [128, HW] out tile per c -> a single 0.8MB DMA, so
                # the ~600ns per-dma_start sequencer cost stays off the
                # critical path (32 small DMAs saturated the Sync queue).
                out_t = out_pool.tile([BPC * K, HW], mybir.dt.bfloat16, tag="out")
                for q in range(NQ):
                    u = c * NQ + q
                    f0 = q * FDQ
                    rep = psum_pool.tile(
                        [BPC * K, NCH * BANK], mybir.dt.float32, tag="rep"
                    )
                    for j in range(NCH):
                        nc.tensor.matmul(
                            rep[:, j * BANK : j * BANK + CH],
                            lhsT=am3[:, HW + c * 128 : HW + (c + 1) * 128],
                            rhs=am3[:, f0 + j * CH : f0 + (j + 1) * CH],
                            start=True,
                            stop=True,
                        )
                    rep3 = rep[:].rearrange("p (j x) -> p j x", j=NCH)[:, :, 0:CH]
                    out_sl = out_t[:, f0 : f0 + FDQ]
                    first_sl = first2[:, f0 : f0 + FDQ]
                    if paths[u] == "B":
                        rsb = rsb_pool.tile(
                            [BPC * K, FDQ], mybir.dt.bfloat16, tag="rsb"
                        )
                        rsb3 = rsb[:].rearrange("p (j x) -> p j x", x=CH)
                        nc.scalar.copy(rsb3, rep3)
                        nc.vector.tensor_mul(out_sl, first_sl, rsb[:])
                    else:
                        out3 = out_sl.rearrange("p (j x) -> p j x", x=CH)
                        f3 = first_sl.rearrange("p (j x) -> p j x", x=CH)
                        nc.vector.tensor_mul(out3, f3, rep3)
                nc.sync.dma_start(out=out_ap[c, :, :], in_=out_t[:])
    nc.compile()
    return nc


def _get_program(repeat=1, **variant):
    key = (repeat, tuple(sorted(variant.items())))
    if key not in _PROGRAMS:
        _PROGRAMS[key] = _build_program(repeat, **variant)
    return _PROGRAMS[key]


def _make_amsel(am_core):
    """am_core [BPC*C, HW] fp32 (row = b*C + c) -> bf16 plane + selectors."""
    import ml_dtypes

    bf16 = ml_dtypes.bfloat16
    m = np.zeros((BPC * C, HW + C * 128), dtype=bf16)
    m[:, :HW] = am_core.astype(bf16)
    for c in range(C):
        for b in range(BPC):
            m[b * C + c, HW + c * 128 + b * K : HW + c * 128 + (b + 1) * K] = 1.0
    return m


def _make_inputs(am_np, first_np):
    import ml_dtypes

    bf16 = ml_dtypes.bfloat16
    in_maps = []
    for i in range(NCORES):
        am_i = am_np[BPC * i : BPC * (i + 1)].reshape(BPC * C, HW)
        fi = first_np[BPC * i : BPC * (i + 1)].reshape(BPC * K, HW)
        in_maps.append(
            {
                "amsel": _make_amsel(am_i),
                "first": np.ascontiguousarray(fi.astype(bf16)),
            }
        )
    return in_maps


def _run(am_np, first_np, **spmd_kwargs):
    from concourse.bass_utils import run_bass_kernel_spmd

    nc = _get_program()
    in_maps = _make_inputs(am_np, first_np)
    return run_bass_kernel_spmd(
        nc, in_maps, core_ids=list(range(NCORES)), **spmd_kwargs
    )


def kernel(am_out, first_out):
    am_np = np.asarray(am_out, dtype=np.float32).reshape(B, C, HW)
    first_np = np.asarray(first_out, dtype=np.float32).reshape(B, K, HW)
    res = _run(am_np, first_np)
    # per-core out is [C, BPC*K, HW] bf16 -> [BPC, C*K, HW]
    cores = [
        np.asarray(res.results[i]["out"])
        .reshape(C, BPC, K, HW)
        .transpose(1, 0, 2, 3)
        .reshape(BPC, C * K, HW)
        for i in range(NCORES)
    ]
    out = np.concatenate(cores, axis=0)
    return out.reshape(B, C * K, H, W).astype(np.float32)


# revision 15
# speedup vs baseline: 1.0177x; 1.0177x over previous
"""ConnectionProductBlock on 8 TRN2 NeuronCores.

out[b, c*K + k, h, w] = am_out[b, c, h, w] * first_out[b, k, h, w]
  with B=16, C=8, K=64, H=W=56.

Data parallel over batch, 2 batches per core, no communication.

Strategy (vs the original fp32 pipeline, which was DVE-bound at 1x):
the correctness budget (rel err 2e-2 L2) is far looser than fp32, so
the whole hot path runs in bf16 (measured L2 ~2.9e-3):
  - Inputs host-cast to bf16; output written as bf16 and host-upcast to
    fp32. Halves HBM/SBUF-port traffic (14.6MB -> 7.4MB per core).
  - am is broadcast across each batch's 64 k-partitions by a selector
    matmul on the TensorEngine into PSUM (fp32): per (c, quarter) unit,
    2 x 392-col chunks at 512-aligned offsets = one 2-bank PSUM tile;
    4 PSUM bufs keep the PE 2-3 units ahead so the ScalarE/VectorE
    consumers run back-to-back (with 2 big bufs the pipeline
    ping-ponged: consumer -> PE refill -> consumer serially, and the
    bursty PE never left its mid p-state).
  - Per unit, one of two paths turns PSUM rep + bf16 first into out:
      'B': ScalarE copies rep PSUM->SBUF bf16 (~(172+784)/1.2 ns), then
           VectorE multiplies all-bf16-SBUF at 2x; the q0+q1 pair of
           each c shares one rsb tile so a single fused FD-1568 multiply
           covers both copies.
      'A': VectorE multiplies straight from PSUM at 1x (~(120+784)/.96).
    Every 4th unit on path A balances ACT (~21us) and DVE (~19us).
  - A tiny ScalarE warm-up copy is issued before everything so the
    one-time activation-table load overlaps the input DMAs.
  - One output DMA per c ([128, 3136] bf16, 0.8MB) on the SP HWDGE
    ring: each dma_start costs ~600ns on the issuing sequencer, so
    few/large DMAs keep that off the critical path while still engaging
    all 16 SBUF ports per transfer.
"""

import numpy as np

B, C, K, H, W = 16, 8, 64, 56, 56
HW = H * W  # 3136
NCORES = 8
BPC = B // NCORES  # 2
NQ = 4  # quarter-units per c
FDQ = HW // NQ  # 784
NCH = 2  # PSUM chunks (banks) per quarter-unit
CH = FDQ // NCH  # 392
BANK = 512  # fp32 slots per PSUM bank

# One entry per (c, quarter) unit: 'B' = ACT copy + DVE 2x; 'A' = DVE 1x.
# 2-bank PSUM tiles / 4 bufs keep the PE 2-3 units ahead so consumers
# run back-to-back; every 4th unit on path A balances ACT vs DVE.
DEFAULT_PATHS = tuple("A" if u % 4 == 3 else "B" for u in range(C * NQ))

_PROGRAMS = {}


def _build_program(repeat=1, paths=DEFAULT_PATHS):
    import contextlib

    import concourse.bacc as bacc
    import concourse.mybir as mybir
    import concourse.tile as tile

    nc = bacc.Bacc("TRN2", debug=False)
    # am bf16 plane with per-c selector blocks appended on the free dim.
    # Partition = b*8 + c. sel[b*8+c, c*128 + b*64 + k] = 1, so
    # sel_c.T @ am3 writes rep[b*64+k, f] = am[b, c, f].
    amsel = nc.dram_tensor(
        "amsel", [BPC * C, HW + C * 128], mybir.dt.bfloat16, kind="ExternalInput"
    )
    first = nc.dram_tensor(
        "first", [BPC * K, HW], mybir.dt.bfloat16, kind="ExternalInput"
    )
    # c-major so each (c, half) DMA is one [128, FDH] slice with adjacent
    # (b, k) partition strides; host transposes back to [BPC, C*K, HW].
    out = nc.dram_tensor(
        "out", [C, BPC * K, HW], mybir.dt.bfloat16, kind="ExternalOutput"
    )

    with tile.TileContext(nc) as tc:
        with (
            tc.tile_pool(name="ins", bufs=1) as ins_pool,
            tc.tile_pool(name="rep", bufs=4, space="PSUM") as psum_pool,
            tc.tile_pool(name="rsb", bufs=3) as rsb_pool,
            tc.tile_pool(name="rsbq", bufs=3) as rsbq_pool,
            tc.tile_pool(name="outs", bufs=3) as out_pool,
            tc.For_i(0, repeat, 1) if repeat > 1 else contextlib.nullcontext(),
        ):
            # ACT warm-up: the first ACTIVATE triggers the ~2.7us
            # activation-table load; spend it on 2 dummy elements while
            # the input DMAs are still in flight.
            warm_a = ins_pool.tile([1, 2], mybir.dt.float32, tag="warm_a")
            warm_b = ins_pool.tile([1, 2], mybir.dt.float32, tag="warm_b")
            nc.vector.memset(warm_a[:], 0.0)
            nc.scalar.copy(warm_b[:], warm_a[:])

            am3 = ins_pool.tile([BPC * C, HW + C * 128], mybir.dt.bfloat16, tag="am3")
            first2 = ins_pool.tile([BPC * K, HW], mybir.dt.bfloat16, tag="first2")
            nc.sync.dma_start(out=am3[:], in_=amsel.ap())
            nc.sync.dma_start(out=first2[:], in_=first.ap())

            out_ap = out.ap()
            for c in range(C):
                # One [128, HW] out tile per c -> a single 0.8MB DMA, so
                # the ~600ns per-dma_start sequencer cost stays off the
                # critical path (32 small DMAs saturated the Sync queue).
                out_t = out_pool.tile([BPC * K, HW], mybir.dt.bfloat16, tag="out")
                # q0+q1 (both path B) share one rsb tile so their two ACT
                # copies feed a single fused FD-1568 DVE multiply.
                rsb2 = rsb_pool.tile(
                    [BPC * K, 2 * FDQ], mybir.dt.bfloat16, tag="rsb"
                )
                for q in range(NQ):
                    u = c * NQ + q
                    f0 = q * FDQ
                    rep = psum_pool.tile(
                        [BPC * K, NCH * BANK], mybir.dt.float32, tag="rep"
                    )
                    for j in range(NCH):
                        nc.tensor.matmul(
                            rep[:, j * BANK : j * BANK + CH],
                            lhsT=am3[:, HW + c * 128 : HW + (c + 1) * 128],
                            rhs=am3[:, f0 + j * CH : f0 + (j + 1) * CH],
                            start=True,
                            stop=True,
                        )
                    rep3 = rep[:].rearrange("p (j x) -> p j x", j=NCH)[:, :, 0:CH]
                    out_sl = out_t[:, f0 : f0 + FDQ]
                    first_sl = first2[:, f0 : f0 + FDQ]
                    if q < 2:
                        rsb_sl = rsb2[:, q * FDQ : (q + 1) * FDQ]
                        rsb3 = rsb_sl.rearrange("p (j x) -> p j x", x=CH)
                        nc.scalar.copy(rsb3, rep3)
                        if q == 1:
                            nc.vector.tensor_mul(
                                out_t[:, 0 : 2 * FDQ],
                                first2[:, 0 : 2 * FDQ],
                                rsb2[:],
                            )
                    elif paths[u] == "B":
                        rsb = rsbq_pool.tile(
                            [BPC * K, FDQ], mybir.dt.bfloat16, tag="rsbq"
                        )
                        rsb3 = rsb[:].rearrange("p (j x) -> p j x", x=CH)
                        nc.scalar.copy(rsb3, rep3)
                        nc.vector.tensor_mul(out_sl, first_sl, rsb[:])
                    else:
                        out3 = out_sl.rearrange("p (j x) -> p j x", x=CH)
                        f3 = first_sl.rearrange("p (j x) -> p j x", x=CH)
                        nc.vector.tensor_mul(out3, f3, rep3)
                nc.sync.dma_start(out=out_ap[c, :, :], in_=out_t[:])
    nc.compile()
    return nc


def _get_program(repeat=1, **variant):
    key = (repeat, tuple(sorted(variant.items())))
    if key not in _PROGRAMS:
        _PROGRAMS[key] = _build_program(repeat, **variant)
    return _PROGRAMS[key]


def _make_amsel(am_core):
    """am_core [BPC*C, HW] fp32 (row = b*C + c) -> bf16 plane + selectors."""
    import ml_dtypes

    bf16 = ml_dtypes.bfloat16
    m = np.zeros((BPC * C, HW + C * 128), dtype=bf16)
    m[:, :HW] = am_core.astype(bf16)
    for c in range(C):
        for b in range(BPC):
            m[b * C + c, HW + c * 128 + b * K : HW + c * 128 + (b + 1) * K] = 1.0
    return m


def _make_inputs(am_np, first_np):
    import ml_dtypes

    bf16 = ml_dtypes.bfloat16
    in_maps = []
    for i in range(NCORES):
        am_i = am_np[BPC * i : BPC * (i + 1)].reshape(BPC * C, HW)
        fi = first_np[BPC * i : BPC * (i + 1)].reshape(BPC * K, HW)
        in_maps.append(
            {
                "amsel": _make_amsel(am_i),
                "first": np.ascontiguousarray(fi.astype(bf16)),
            }
        )
    return in_maps


def _run(am_np, first_np, **spmd_kwargs):
    from concourse.bass_utils import run_bass_kernel_spmd

    nc = _get_program()
    in_maps = _make_inputs(am_np, first_np)
    return run_bass_kernel_spmd(
        nc, in_maps, core_ids=list(range(NCORES)), **spmd_kwargs
    )


def kernel(am_out, first_out):
    am_np = np.asarray(am_out, dtype=np.float32).reshape(B, C, HW)
    first_np = np.asarray(first_out, dtype=np.float32).reshape(B, K, HW)
    res = _run(am_np, first_np)
    # per-core out is [C, BPC*K, HW] bf16 -> [BPC, C*K, HW]
    cores = [
        np.asarray(res.results[i]["out"])
        .reshape(C, BPC, K, HW)
        .transpose(1, 0, 2, 3)
        .reshape(BPC, C * K, HW)
        for i in range(NCORES)
    ]
    out = np.concatenate(cores, axis=0)
    return out.reshape(B, C * K, H, W).astype(np.float32)
